# revision 62
# baseline (speedup 1.0000x reference)
"""Trainium2 Bass kernel: batched single-head causal attention.

Problem: x [8, 2048, 1024] f32; Wq/Wk/Wv [64, 1024] f32.
  Q = x @ Wq.T; K = x @ Wk.T; V = x @ Wv.T            (per batch)
  out = softmax(mask(Q K^T / sqrt(1024))) @ V          -> [8, 2048, 64]

Sharding: data-parallel over batch B=8 across the 8 NeuronCores (one batch
element per core); the small weights are replicated.

Per-core algorithm (T=2048, C=1024, H=64), all matmul operands bf16
(accumulation in fp32 PSUM; verified rel-err ~1e-3 vs the fp32 reference,
comfortably under the 2e-2 gate):
  - x is cast to bf16 on host and transposed DRAM->SBUF by the DMA XBAR
    engine (InstDmaTransposeAnt, 16x128 tiles) into xT [128, 8k, T],
    sliced per (C-chunk, tq-block) so projections can start early.  This
    removes all PE transposes of x and their PSUM->SBUF copies.
  - Projections in transposed layout: QT [64, T] (scaled by 1/32, folded
    into Wq on host) and stacked KVT [128, T] (KT rows 0:64, VT 64:128).
  - V re-transposed to natural V_aug [T, 65] (PE transpose w/ bf16
    identity) with a ones column so attention row-sums fall out of the
    attn @ V matmul.
  - Scores computed TRANSPOSED: sT[tk, tq] = K Q^T chunks [128, 512];
    fully-masked blocks skipped; softmax max-subtraction skipped (|s| <~
    1.5); exp on ACT straight out of PSUM (bf16 out); early (off-diagonal)
    chunks are issued as row-packed pairs (PE rows 0:64 / 64:128 via
    base-64 SBUF duplicates of QT/KT) sharing one exp instruction over a
    2-bank PSUM pair tile; diagonal chunks are column-trimmed and
    multiplied by precomputed 0/1 bf16 masks.
  - outT_aug [65, 512] = V_aug^T @ expT accumulated over tk chunks; final
    PE transpose back to natural [128, 65] chunks (fp32); DVE reciprocal +
    tensor_scalar multiply normalizes; DMA out.
"""

import numpy as np

import concourse.bass as bass
import concourse.mybir as mybir
import concourse.tile as tile
from concourse import bacc
from concourse.bass_utils import run_bass_kernel_spmd

B = 8
T = 2048
C = 1024
H = 64
P = 128
NT = T // P   # 16 row chunks
NCH = C // P  # 8 contraction chunks
NB = 4        # tq blocks
BQ = 512      # tq block size
F32 = mybir.dt.float32
BF16 = mybir.dt.bfloat16

# One exp instruction per row-packed early score pair (2-bank PSUM tiles).
EXP_PAIR = True
# Issue early score chunks as row-packed pairs (PE rows 0:64 / 64:128 via
# base-64 SBUF duplicates of QT/KT).  False = plain K=64 matmuls at base 0,
# no duplicates needed.  HW-measured: unpaired K=64 bf16 matmuls run far
# below full rate (131us vs 92us whole-kernel) — keep pairing on.
PAIR_SCORES = True


def declare_io(nc):
    """DRAM tensors; shared by kernel() and the looped timing harness."""
    return {
        "x": nc.dram_tensor("x", [T, C], BF16, kind="ExternalInput").ap(),
        "w1": nc.dram_tensor("w1", [C, 128], BF16, kind="ExternalInput").ap(),
        "w2": nc.dram_tensor("w2", [C, 128], BF16, kind="ExternalInput").ap(),
        "masks": nc.dram_tensor("masks", [P, P], BF16, kind="ExternalInput").ap(),
        "identb": nc.dram_tensor("identb", [P, 256], BF16, kind="ExternalInput").ap(),
        "ident": nc.dram_tensor("ident", [P, P], F32, kind="ExternalInput").ap(),
        "out": nc.dram_tensor("out", [T, H], F32, kind="ExternalOutput").ap(),
    }


def build_nc():
    nc = bacc.Bacc("TRN2", target_bir_lowering=False)
    io = declare_io(nc)
    with tile.TileContext(nc) as tc:
        _emit(nc, tc, io)
    nc.compile()
    return nc


def _emit(nc, tc, io):
    import contextlib

    x_d, w1_d, w2_d, m_d = io["x"], io["w1"], io["w2"], io["masks"]
    ib_d, i_d, o_d = io["identb"], io["ident"], io["out"]

    ctx = contextlib.ExitStack()
    with ctx:
        consts = ctx.enter_context(tc.tile_pool(name="consts", bufs=1))
        persist = ctx.enter_context(tc.tile_pool(name="persist", bufs=1))
        expp = ctx.enter_context(tc.tile_pool(name="expp", bufs=4))
        oaugp = ctx.enter_context(tc.tile_pool(name="oaugp", bufs=2))
        outp = ctx.enter_context(tc.tile_pool(name="outp", bufs=4))
        recp = ctx.enter_context(tc.tile_pool(name="recp", bufs=4))
        # PSUM: psP 3 banks + psC 2x2 banks + psT 1 bank = 8 banks exactly
        psP = ctx.enter_context(tc.tile_pool(name="psP", bufs=3, space="PSUM"))
        psC = ctx.enter_context(tc.tile_pool(name="psC", bufs=2, space="PSUM"))
        psT = ctx.enter_context(tc.tile_pool(name="psT", bufs=1, space="PSUM"))

        # ---- constants ----
        # All consts ride HWDGE (the SWDGE path has ~4-5us latency).  They
        # are emitted interleaved with the x transposes on the sync queue so
        # the startup window prioritizes the first tq-block's data: first
        # x slice, wq, rest of tr=0, wkv, then the remaining transposes,
        # then the late-needed identities/masks.
        w1_sb = consts.tile([P, NCH, P], BF16, tag="w1")
        w2_sb = consts.tile([P, NCH, P], BF16, tag="w2")
        identb_sb = consts.tile([P, 256], BF16, tag="identb")
        masks_sb = consts.tile([P, P], BF16, tag="masks")
        ident_sb = consts.tile([P, P], F32, tag="ident")

        # ---- persistent tiles ----
        xT = persist.tile([P, NCH, T], BF16, tag="xT")   # xT[p,k,t] = x[t, k*128+p]
        qt = persist.tile([64, T], BF16, tag="qt")       # QT (pre-scaled by 1/32)
        kvt = persist.tile([P, T], BF16, tag="kvt")      # rows 0:64 KT, 64:128 VT
        vaug = persist.tile([P, NT, H + 1], BF16, tag="vaug")  # V chunks + ones col
        # Base-64 duplicates of QT/KT: K=64 matmuls stream at half rate
        # (moving fetch uses only 64 partitions), so early score chunks are
        # issued as row-packed pairs -- the odd member needs both operands in
        # partitions 64:128.  SBUF->SBUF DMA shifts partitions.
        qt64 = persist.tile([P, T], BF16, tag="qt64")    # rows 64:128 = QT
        kt64 = persist.tile([P, T], BF16, tag="kt64")    # rows 64:128 = KT

        nc.vector.memset(vaug[:, :, H : H + 1], 1.0)

        # ---- x DMA transposes: DRAM -> xT, sliced (T-range, C-chunk) ----
        # T split 512/512/1024: small first slices so block-0/1 compute
        # starts early; the tail in bigger ops to bound HWDGE overhead.
        def xpose(t0, t1, k):
            nc.sync.dma_start(
                out=xT[:, k, t0:t1],
                in_=x_d[t0:t1, k * P : (k + 1) * P],
                transpose=True,
            )

        xpose(0, 1024, 0)
        nc.sync.dma_start(
            out=w1_sb, in_=w1_d.rearrange("(k p) m -> p k m", p=P)
        )
        nc.sync.dma_start(
            out=w2_sb, in_=w2_d.rearrange("(k p) m -> p k m", p=P)
        )
        xpose(0, 1024, 1)
        nc.sync.dma_start(out=masks_sb, in_=m_d)
        nc.sync.dma_start(out=identb_sb, in_=ib_d)
        for k in range(2, NCH):
            xpose(0, 1024, k)
        for k in range(NCH):
            xpose(1024, 2048, k)
        nc.sync.dma_start(out=ident_sb, in_=i_d)

        # lag pipeline of chunk-wise score -> exp/mask -> AV matmul
        pending = []

        def flush_av(limit):
            while len(pending) > limit:
                av_t, ex_ap, i_, last_ = pending.pop(0)
                nc.tensor.matmul(
                    av_t,
                    lhsT=vaug[:, i_, 0 : H + 1],
                    rhs=ex_ap,
                    start=(i_ == 0),
                    stop=last_,
                )

        def c_score(n, i, hi=False, sp=None, plane=None):
            """Score matmul for chunk i of block n.  hi=True issues it in PE
            rows 64:128 (reading the base-64 QT/KT duplicates) so it runs
            concurrently with the preceding hi=False chunk."""
            d = i - 4 * n
            off = 128 * d if d > 0 else 0
            if sp is None:
                spt = psC.tile([P, 2, BQ], F32, tag="psc")
                sp = spt[:, 0, :]
                dst = sp[:, off:BQ]
            else:
                dst = sp[:, plane, off:BQ]
            if hi and PAIR_SCORES:
                nc.tensor.matmul(
                    dst,
                    lhsT=kt64[64:128, i * P : (i + 1) * P],
                    rhs=qt64[64:128, n * BQ + off : (n + 1) * BQ],
                    start=True,
                    stop=True,
                    tile_position=(64, 0),
                )
            else:
                nc.tensor.matmul(
                    dst,
                    lhsT=kvt[0:64, i * P : (i + 1) * P],
                    rhs=qt[:, n * BQ + off : (n + 1) * BQ],
                    start=True,
                    stop=True,
                )
            return sp, off

        def c_chunk(av, n, i, nchunks):
            """Diagonal chunk: trimmed score -> +mask bias -> exp -> AV.
            Only columns [off, off+128) need masking (for j >= off+128,
            p + 128*d <= 127 + 128*d < j always holds) and within that
            window the pattern is the same lower-triangle for every d, so a
            single PE matmul (identity stationary, 0/-1000 bias moving)
            accumulates the mask into the score PSUM; exp of -1000 is
            exactly 0.  Keeps the whole chain on PE+ACT (no DVE hop)."""
            d = i - 4 * n
            off = 128 * d if d > 0 else 0
            spt = psC.tile([P, 2, BQ], F32, tag="psc")
            sp = spt[:, 0, :]
            nc.tensor.matmul(
                sp[:, off:BQ],
                lhsT=kvt[0:64, i * P : (i + 1) * P],
                rhs=qt[:, n * BQ + off : (n + 1) * BQ],
                start=True,
                stop=False,
            )
            nc.tensor.matmul(
                sp[:, off : off + P],
                lhsT=identb_sb[:, 0:128],
                rhs=masks_sb,
                start=False,
                stop=True,
            )
            ex = expp.tile([P, BQ], BF16, tag="ex")
            nc.scalar.activation(
                out=ex[:, off:BQ],
                in_=sp[:, off:BQ],
                func=mybir.ActivationFunctionType.Exp,
            )
            pending.append((av[0:65, off:BQ], ex[:, off:BQ], i, i == nchunks - 1))
            flush_av(2)

        def c_pair(av, n, i, nchunks):
            """Row-packed early score pair sharing one [P, 2, BQ] PSUM tile
            and a single exp instruction over both halves."""
            assert i + 1 < 4 * n, "pairs are for early (unmasked) chunks"
            if EXP_PAIR:
                sp = psC.tile([P, 2, BQ], F32, tag="psc")
                c_score(n, i, hi=False, sp=sp, plane=0)
                c_score(n, i + 1, hi=True, sp=sp, plane=1)
                ex = expp.tile([P, 2, BQ], BF16, tag="ex2")
                nc.scalar.activation(
                    out=ex, in_=sp, func=mybir.ActivationFunctionType.Exp
                )
                pending.append((av[0:65, :], ex[:, 0, :], i, False))
                pending.append(
                    (av[0:65, :], ex[:, 1, :], i + 1, i + 1 == nchunks - 1)
                )
            else:
                sp1, _ = c_score(n, i, hi=False)
                sp2, _ = c_score(n, i + 1, hi=True)
                for sp_, ii in ((sp1, i), (sp2, i + 1)):
                    ex = expp.tile([P, BQ], BF16, tag="ex")
                    nc.scalar.activation(
                        out=ex, in_=sp_, func=mybir.ActivationFunctionType.Exp
                    )
                    pending.append((av[0:65, :], ex, ii, ii == nchunks - 1))
            flush_av(2)

        for n in range(NB):
            nchunks = 4 * (n + 1)

            # ---- projections for tq block n: pass1 = [K|Q], pass2 = [Q|V],
            # interleaved per C-chunk so both accumulations track arriving
            # xT slices.  The halves land exactly where consumers need them:
            # K@0:64 (score-lo lhsT), Q@64:128 (score-hi rhs), Q@0:64
            # (score-lo rhs), V@64:128 (V transposes) — all same-partition
            # PSUM->SBUF copies, no SBUF->SBUF DMAs. ----
            ps1 = psP.tile([P, BQ], F32, tag="psp")
            ps2 = psP.tile([P, BQ], F32, tag="psp")
            for k in range(NCH):
                nc.tensor.matmul(
                    ps1,
                    lhsT=w1_sb[:, k, :],
                    rhs=xT[:, k, n * BQ : (n + 1) * BQ],
                    start=(k == 0),
                    stop=(k == NCH - 1),
                )
                nc.tensor.matmul(
                    ps2,
                    lhsT=w2_sb[:, k, :],
                    rhs=xT[:, k, n * BQ : (n + 1) * BQ],
                    start=(k == 0),
                    stop=(k == NCH - 1),
                )
            # copies split ACT/DVE so the pair/diag consumers unblock fast:
            # qt gates pair-lo rhs, qt64 gates pair-hi rhs, kvt0 gates the
            # diag lhsT (later), kvtV gates the V transposes.
            nc.scalar.copy(out=qt[:, n * BQ : (n + 1) * BQ], in_=ps2[0:64, :])
            if PAIR_SCORES and n >= 1:
                nc.vector.tensor_copy(
                    out=qt64[64:128, n * BQ : (n + 1) * BQ], in_=ps1[64:128, :]
                )
            nc.vector.tensor_copy(
                out=kvt[0:64, n * BQ : (n + 1) * BQ], in_=ps1[0:64, :]
            )
            nc.vector.tensor_copy(
                out=kvt[64:128, n * BQ : (n + 1) * BQ], in_=ps2[64:128, :]
            )

            # ---- early chunks (pairs): depend only on OLD kvt/vaug ----
            # AV lag 2: PE issues the next pair's scores before the previous
            # pair's AV matmuls, hiding the exp latency.
            av = psP.tile([65, BQ], F32, tag="psp")
            for i in range(0, 4 * n, 2):
                c_pair(av, n, i, nchunks)

            # ---- V natural chunks for this block (bf16 PE transposes) ----
            for j in range(4 * n, 4 * n + 4):
                vp = psT.tile([P, H], BF16, tag="pst")
                nc.tensor.transpose(
                    out=vp,
                    in_=kvt[64:128, j * P : (j + 1) * P],
                    identity=identb_sb[64:128, 64:128],
                )
                nc.vector.tensor_copy(out=vaug[:, j, 0:H], in_=vp)

            # ---- diagonal chunks ----
            for i in range(4 * n, nchunks):
                c_chunk(av, n, i, nchunks)
            flush_av(0)

            # Base-64 duplicate of KT for LATER blocks' pair-hi lhsT, via a
            # PE "shift matmul": a +64-shifted identity as the stationary
            # operand writes rows 64:128 of a PSUM bank, then a
            # same-partition DVE copy lands it in SBUF.  (SBUF->SBUF DMAs
            # are out: HWDGE ones race the x DMA-transposes — HW data
            # hazard — and SWDGE ones have ~3-5us latency.)  Emitted after
            # the diag section: its consumer is a block away.
            if PAIR_SCORES and n <= 2:
                shk = psC.tile([P, 2, BQ], F32, tag="psc")
                nc.tensor.matmul(
                    shk[:, 0, :],
                    lhsT=identb_sb[0:64, 128:256],
                    rhs=kvt[0:64, n * BQ : (n + 1) * BQ],
                    start=True,
                    stop=True,
                )
                nc.vector.tensor_copy(
                    out=kt64[64:128, n * BQ : (n + 1) * BQ],
                    in_=shk[64:128, 0, :],
                )

            # ---- transpose back (bf16), normalize (f32), store ----
            oa = oaugp.tile([65, BQ], BF16, tag="oa")
            nc.vector.tensor_copy(out=oa, in_=av)
            tpt = psC.tile([P, 2, BQ], F32, tag="psc")
            tp = tpt[:, 0, 0:288].bitcast(BF16).rearrange(
                "p (q c) -> p q c", c=144
            )
            for q in range(4):
                nc.tensor.transpose(
                    out=tp[:, q, 0:65],
                    in_=oa[:, q * P : (q + 1) * P],
                    identity=identb_sb[0:65, 0:65],
                )
            r = recp.tile([P, 4], F32, tag="r")
            nc.vector.reciprocal(r, tp[:, :, 64])
            ot = outp.tile([P, 4, H], F32, tag="ot")
            for q in range(4):
                nc.vector.tensor_scalar_mul(
                    ot[:, q, :], tp[:, q, 0:64], r[:, q : q + 1]
                )
            # SBUF->DRAM store on the sync HWDGE queue (free after the x
            # transposes; keeps the serial SWDGE queue for the qt64/kt64
            # duplicates so they don't park behind these stores).
            nc.sync.dma_start(
                out=o_d[n * BQ : (n + 1) * BQ, :].rearrange(
                    "(q p) h -> p q h", p=P
                ),
                in_=ot,
            )


def host_inputs(Wq, Wk, Wv):
    """Replicated per-core constant inputs from the raw weights."""
    bf = mybir.dt.np(BF16)
    scale = np.float32(1.0 / np.sqrt(np.float32(C)))
    w1 = np.empty((C, 128), dtype=np.float32)
    w1[:, 0:64] = Wk.T
    w1[:, 64:128] = Wq.T * scale
    w2 = np.empty((C, 128), dtype=np.float32)
    w2[:, 0:64] = Wq.T * scale
    w2[:, 64:128] = Wv.T
    p = np.arange(P, dtype=np.int64)[:, None]
    j = np.arange(P, dtype=np.int64)[None, :]
    masks = np.where(p <= j, 0.0, -1000.0).astype(np.float32)
    identb = np.concatenate(
        [np.eye(P, dtype=np.float32), np.eye(P, P, 64, dtype=np.float32)],
        axis=1,
    )
    return (
        w1.astype(bf),
        w2.astype(bf),
        masks.astype(bf),
        identb.astype(bf),
        np.eye(P, dtype=np.float32),
    )


def kernel(x, Wq, Wk, Wv):
    x = np.asarray(x, dtype=np.float32)
    Wq = np.asarray(Wq, dtype=np.float32)
    Wk = np.asarray(Wk, dtype=np.float32)
    Wv = np.asarray(Wv, dtype=np.float32)
    assert x.shape == (B, T, C), x.shape

    bf = mybir.dt.np(BF16)
    xb = np.ascontiguousarray(x.astype(bf))
    w1, w2, masks, identb, ident = host_inputs(Wq, Wk, Wv)
    nc = build_nc()
    in_maps = [
        {
            "x": np.ascontiguousarray(xb[b]),
            "w1": w1,
            "w2": w2,
            "masks": masks,
            "identb": identb,
            "ident": ident,
        }
        for b in range(B)
    ]
    try:
        res = run_bass_kernel_spmd(nc, in_maps, core_ids=list(range(B)))
    except Exception:
        # transient device/mesh hiccups happen through the tunnel; one retry
        res = run_bass_kernel_spmd(nc, in_maps, core_ids=list(range(B)))
    return np.stack([res.results[b]["out"] for b in range(B)], axis=0)


# revision 63
# speedup vs baseline: 3.1820x; 3.1820x over previous
"""Trainium2 Bass kernel: batched single-head causal attention.

Problem: x [8, 2048, 1024] f32; Wq/Wk/Wv [64, 1024] f32.
  Q = x @ Wq.T; K = x @ Wk.T; V = x @ Wv.T            (per batch)
  out = softmax(mask(Q K^T / sqrt(1024))) @ V          -> [8, 2048, 64]

Sharding: data-parallel over batch B=8 across the 8 NeuronCores (one batch
element per core); the small weights are replicated.

Per-core algorithm (T=2048, C=1024, H=64), all matmul operands bf16
(accumulation in fp32 PSUM; verified rel-err ~1e-3 vs the fp32 reference,
comfortably under the 2e-2 gate):
  - x is cast to bf16 on host and transposed DRAM->SBUF by the DMA XBAR
    engine (InstDmaTransposeAnt, 16x128 tiles) into xT [128, 8k, T],
    sliced per (C-chunk, tq-block) so projections can start early.  This
    removes all PE transposes of x and their PSUM->SBUF copies.
  - Projections in transposed layout: QT [64, T] (scaled by 1/32, folded
    into Wq on host) and stacked KVT [128, T] (KT rows 0:64, VT 64:128).
  - V re-transposed to natural V_aug [T, 65] (PE transpose w/ bf16
    identity) with a ones column so attention row-sums fall out of the
    attn @ V matmul.
  - Scores computed TRANSPOSED: sT[tk, tq] = K Q^T chunks [128, 512];
    fully-masked blocks skipped; softmax max-subtraction skipped (|s| <~
    1.5); exp on ACT straight out of PSUM (bf16 out); early (off-diagonal)
    chunks are issued as row-packed pairs (PE rows 0:64 / 64:128 via
    base-64 SBUF duplicates of QT/KT) sharing one exp instruction over a
    2-bank PSUM pair tile; diagonal chunks are column-trimmed and
    multiplied by precomputed 0/1 bf16 masks.
  - outT_aug [65, 512] = V_aug^T @ expT accumulated over tk chunks; final
    PE transpose back to natural [128, 65] chunks (fp32); DVE reciprocal +
    tensor_scalar multiply normalizes; DMA out.
"""

import numpy as np

import concourse.bass as bass
import concourse.mybir as mybir
import concourse.tile as tile
from concourse import bacc
from concourse.bass_utils import run_bass_kernel_spmd

B = 8
T = 2048
C = 1024
H = 64
P = 128
NT = T // P   # 16 row chunks
NCH = C // P  # 8 contraction chunks
NB = 4        # tq blocks
BQ = 512      # tq block size
F32 = mybir.dt.float32
BF16 = mybir.dt.bfloat16

# One exp instruction per row-packed early score pair (2-bank PSUM tiles).
EXP_PAIR = True
# Issue early score chunks as row-packed pairs (PE rows 0:64 / 64:128 via
# base-64 SBUF duplicates of QT/KT).  False = plain K=64 matmuls at base 0,
# no duplicates needed.  HW-measured: unpaired K=64 bf16 matmuls run far
# below full rate (131us vs 92us whole-kernel) — keep pairing on.
PAIR_SCORES = True


def declare_io(nc):
    """DRAM tensors; shared by kernel() and the looped timing harness."""
    return {
        "x": nc.dram_tensor("x", [T, C], BF16, kind="ExternalInput").ap(),
        "w1": nc.dram_tensor("w1", [C, 128], BF16, kind="ExternalInput").ap(),
        "w2": nc.dram_tensor("w2", [C, 128], BF16, kind="ExternalInput").ap(),
        "masks": nc.dram_tensor("masks", [P, P], BF16, kind="ExternalInput").ap(),
        "identb": nc.dram_tensor("identb", [P, 256], BF16, kind="ExternalInput").ap(),
        "ident": nc.dram_tensor("ident", [P, P], F32, kind="ExternalInput").ap(),
        "out": nc.dram_tensor("out", [T, H], F32, kind="ExternalOutput").ap(),
    }


def build_nc():
    nc = bacc.Bacc("TRN2", target_bir_lowering=False)
    io = declare_io(nc)
    with tile.TileContext(nc) as tc:
        _emit(nc, tc, io)
    nc.compile()
    return nc


def _emit(nc, tc, io):
    import contextlib

    x_d, w1_d, w2_d, m_d = io["x"], io["w1"], io["w2"], io["masks"]
    ib_d, i_d, o_d = io["identb"], io["ident"], io["out"]

    ctx = contextlib.ExitStack()
    with ctx:
        consts = ctx.enter_context(tc.tile_pool(name="consts", bufs=1))
        persist = ctx.enter_context(tc.tile_pool(name="persist", bufs=1))
        expp = ctx.enter_context(tc.tile_pool(name="expp", bufs=4))
        oaugp = ctx.enter_context(tc.tile_pool(name="oaugp", bufs=2))
        outp = ctx.enter_context(tc.tile_pool(name="outp", bufs=4))
        recp = ctx.enter_context(tc.tile_pool(name="recp", bufs=4))
        # PSUM: psP 3 banks + psC 2x2 banks + psT 1 bank = 8 banks exactly
        psP = ctx.enter_context(tc.tile_pool(name="psP", bufs=3, space="PSUM"))
        psC = ctx.enter_context(tc.tile_pool(name="psC", bufs=2, space="PSUM"))
        psT = ctx.enter_context(tc.tile_pool(name="psT", bufs=1, space="PSUM"))

        # ---- constants ----
        # All consts ride HWDGE (the SWDGE path has ~4-5us latency).  They
        # are emitted interleaved with the x transposes on the sync queue so
        # the startup window prioritizes the first tq-block's data: first
        # x slice, wq, rest of tr=0, wkv, then the remaining transposes,
        # then the late-needed identities/masks.
        w1_sb = consts.tile([P, NCH, P], BF16, tag="w1")
        w2_sb = consts.tile([P, NCH, P], BF16, tag="w2")
        identb_sb = consts.tile([P, 256], BF16, tag="identb")
        masks_sb = consts.tile([P, P], BF16, tag="masks")
        ident_sb = consts.tile([P, P], F32, tag="ident")

        # ---- persistent tiles ----
        xT = persist.tile([P, NCH, T], BF16, tag="xT")   # xT[p,k,t] = x[t, k*128+p]
        qt = persist.tile([64, T], BF16, tag="qt")       # QT (pre-scaled by 1/32)
        kvt = persist.tile([P, T], BF16, tag="kvt")      # rows 0:64 KT, 64:128 VT
        vaug = persist.tile([P, NT, H + 1], BF16, tag="vaug")  # V chunks + ones col
        # Base-64 duplicates of QT/KT: K=64 matmuls stream at half rate
        # (moving fetch uses only 64 partitions), so early score chunks are
        # issued as row-packed pairs -- the odd member needs both operands in
        # partitions 64:128.  SBUF->SBUF DMA shifts partitions.
        qt64 = persist.tile([P, T], BF16, tag="qt64")    # rows 64:128 = QT
        kt64 = persist.tile([P, T], BF16, tag="kt64")    # rows 64:128 = KT

        nc.vector.memset(vaug[:, :, H : H + 1], 1.0)

        # ---- x DMA transposes: DRAM -> xT, sliced (T-range, C-chunk) ----
        # T split 512/512/1024: small first slices so block-0/1 compute
        # starts early; the tail in bigger ops to bound HWDGE overhead.
        def xpose(t0, t1, k):
            nc.sync.dma_start(
                out=xT[:, k, t0:t1],
                in_=x_d[t0:t1, k * P : (k + 1) * P],
                transpose=True,
            )

        # Emission order on the sync queue: x transposes first, const loads
        # at the tail.  Under the For_i timing loop the const tiles are
        # reloaded every iteration, and a const DMA placed mid-stream chains
        # the NEXT iteration's x feed behind this iteration's last const
        # readers (WAR) — serializing iterations.  At the tail they only
        # relay behind the previous iteration's matching consumers.
        xpose(0, 1024, 0)
        nc.sync.dma_start(
            out=w1_sb, in_=w1_d.rearrange("(k p) m -> p k m", p=P)
        )
        nc.sync.dma_start(
            out=w2_sb, in_=w2_d.rearrange("(k p) m -> p k m", p=P)
        )
        for k in range(1, NCH):
            xpose(0, 1024, k)
        nc.sync.dma_start(out=masks_sb, in_=m_d)
        nc.sync.dma_start(out=identb_sb, in_=ib_d)
        for k in range(NCH):
            xpose(1024, 2048, k)
        nc.sync.dma_start(out=ident_sb, in_=i_d)

        # lag pipeline of chunk-wise score -> exp/mask -> AV matmul
        pending = []

        def flush_av(limit):
            while len(pending) > limit:
                av_t, ex_ap, i_, last_ = pending.pop(0)
                nc.tensor.matmul(
                    av_t,
                    lhsT=vaug[:, i_, 0 : H + 1],
                    rhs=ex_ap,
                    start=(i_ == 0),
                    stop=last_,
                )

        def c_score(n, i, hi=False, sp=None, plane=None):
            """Score matmul for chunk i of block n.  hi=True issues it in PE
            rows 64:128 (reading the base-64 QT/KT duplicates) so it runs
            concurrently with the preceding hi=False chunk."""
            d = i - 4 * n
            off = 128 * d if d > 0 else 0
            if sp is None:
                spt = psC.tile([P, 2, BQ], F32, tag="psc")
                sp = spt[:, 0, :]
                dst = sp[:, off:BQ]
            else:
                dst = sp[:, plane, off:BQ]
            if hi and PAIR_SCORES:
                nc.tensor.matmul(
                    dst,
                    lhsT=kt64[64:128, i * P : (i + 1) * P],
                    rhs=qt64[64:128, n * BQ + off : (n + 1) * BQ],
                    start=True,
                    stop=True,
                    tile_position=(64, 0),
                )
            else:
                nc.tensor.matmul(
                    dst,
                    lhsT=kvt[0:64, i * P : (i + 1) * P],
                    rhs=qt[:, n * BQ + off : (n + 1) * BQ],
                    start=True,
                    stop=True,
                )
            return sp, off

        def c_chunk(av, n, i, nchunks):
            """Diagonal chunk: trimmed score -> +mask bias -> exp -> AV.
            Only columns [off, off+128) need masking (for j >= off+128,
            p + 128*d <= 127 + 128*d < j always holds) and within that
            window the pattern is the same lower-triangle for every d, so a
            single PE matmul (identity stationary, 0/-1000 bias moving)
            accumulates the mask into the score PSUM; exp of -1000 is
            exactly 0.  Keeps the whole chain on PE+ACT (no DVE hop)."""
            d = i - 4 * n
            off = 128 * d if d > 0 else 0
            spt = psC.tile([P, 2, BQ], F32, tag="psc")
            sp = spt[:, 0, :]
            nc.tensor.matmul(
                sp[:, off:BQ],
                lhsT=kvt[0:64, i * P : (i + 1) * P],
                rhs=qt[:, n * BQ + off : (n + 1) * BQ],
                start=True,
                stop=False,
            )
            nc.tensor.matmul(
                sp[:, off : off + P],
                lhsT=identb_sb[:, 0:128],
                rhs=masks_sb,
                start=False,
                stop=True,
            )
            ex = expp.tile([P, BQ], BF16, tag="ex")
            nc.scalar.activation(
                out=ex[:, off:BQ],
                in_=sp[:, off:BQ],
                func=mybir.ActivationFunctionType.Exp,
            )
            pending.append((av[0:65, off:BQ], ex[:, off:BQ], i, i == nchunks - 1))
            flush_av(2)

        def c_pair(av, n, i, nchunks):
            """Row-packed early score pair sharing one [P, 2, BQ] PSUM tile
            and a single exp instruction over both halves."""
            assert i + 1 < 4 * n, "pairs are for early (unmasked) chunks"
            if EXP_PAIR:
                sp = psC.tile([P, 2, BQ], F32, tag="psc")
                c_score(n, i, hi=False, sp=sp, plane=0)
                c_score(n, i + 1, hi=True, sp=sp, plane=1)
                ex = expp.tile([P, 2, BQ], BF16, tag="ex2")
                nc.scalar.activation(
                    out=ex, in_=sp, func=mybir.ActivationFunctionType.Exp
                )
                pending.append((av[0:65, :], ex[:, 0, :], i, False))
                pending.append(
                    (av[0:65, :], ex[:, 1, :], i + 1, i + 1 == nchunks - 1)
                )
            else:
                sp1, _ = c_score(n, i, hi=False)
                sp2, _ = c_score(n, i + 1, hi=True)
                for sp_, ii in ((sp1, i), (sp2, i + 1)):
                    ex = expp.tile([P, BQ], BF16, tag="ex")
                    nc.scalar.activation(
                        out=ex, in_=sp_, func=mybir.ActivationFunctionType.Exp
                    )
                    pending.append((av[0:65, :], ex, ii, ii == nchunks - 1))
            flush_av(2)

        for n in range(NB):
            nchunks = 4 * (n + 1)

            # ---- projections for tq block n: pass1 = [K|Q], pass2 = [Q|V],
            # interleaved per C-chunk so both accumulations track arriving
            # xT slices.  The halves land exactly where consumers need them:
            # K@0:64 (score-lo lhsT), Q@64:128 (score-hi rhs), Q@0:64
            # (score-lo rhs), V@64:128 (V transposes) — all same-partition
            # PSUM->SBUF copies, no SBUF->SBUF DMAs. ----
            ps1 = psP.tile([P, BQ], F32, tag="psp")
            ps2 = psP.tile([P, BQ], F32, tag="psp")
            for k in range(NCH):
                nc.tensor.matmul(
                    ps1,
                    lhsT=w1_sb[:, k, :],
                    rhs=xT[:, k, n * BQ : (n + 1) * BQ],
                    start=(k == 0),
                    stop=(k == NCH - 1),
                )
                nc.tensor.matmul(
                    ps2,
                    lhsT=w2_sb[:, k, :],
                    rhs=xT[:, k, n * BQ : (n + 1) * BQ],
                    start=(k == 0),
                    stop=(k == NCH - 1),
                )
            # copies split ACT/DVE so the pair/diag consumers unblock fast:
            # qt gates pair-lo rhs, qt64 gates pair-hi rhs, kvt0 gates the
            # diag lhsT (later), kvtV gates the V transposes.
            nc.scalar.copy(out=qt[:, n * BQ : (n + 1) * BQ], in_=ps2[0:64, :])
            if PAIR_SCORES and n >= 1:
                nc.vector.tensor_copy(
                    out=qt64[64:128, n * BQ : (n + 1) * BQ], in_=ps1[64:128, :]
                )
            nc.vector.tensor_copy(
                out=kvt[0:64, n * BQ : (n + 1) * BQ], in_=ps1[0:64, :]
            )
            nc.vector.tensor_copy(
                out=kvt[64:128, n * BQ : (n + 1) * BQ], in_=ps2[64:128, :]
            )

            # ---- early chunks (pairs): depend only on OLD kvt/vaug ----
            # AV lag 2: PE issues the next pair's scores before the previous
            # pair's AV matmuls, hiding the exp latency.
            av = psP.tile([65, BQ], F32, tag="psp")
            for i in range(0, 4 * n, 2):
                c_pair(av, n, i, nchunks)

            # ---- V natural chunks for this block (bf16 PE transposes) ----
            for j in range(4 * n, 4 * n + 4):
                vp = psT.tile([P, H], BF16, tag="pst")
                nc.tensor.transpose(
                    out=vp,
                    in_=kvt[64:128, j * P : (j + 1) * P],
                    identity=identb_sb[64:128, 64:128],
                )
                nc.vector.tensor_copy(out=vaug[:, j, 0:H], in_=vp)

            # ---- diagonal chunks ----
            for i in range(4 * n, nchunks):
                c_chunk(av, n, i, nchunks)
            flush_av(0)

            # Base-64 duplicate of KT for LATER blocks' pair-hi lhsT, via a
            # PE "shift matmul": a +64-shifted identity as the stationary
            # operand writes rows 64:128 of a PSUM bank, then a
            # same-partition DVE copy lands it in SBUF.  (SBUF->SBUF DMAs
            # are out: HWDGE ones race the x DMA-transposes — HW data
            # hazard — and SWDGE ones have ~3-5us latency.)  Emitted after
            # the diag section: its consumer is a block away.
            if PAIR_SCORES and n <= 2:
                shk = psC.tile([P, 2, BQ], F32, tag="psc")
                nc.tensor.matmul(
                    shk[:, 0, :],
                    lhsT=identb_sb[0:64, 128:256],
                    rhs=kvt[0:64, n * BQ : (n + 1) * BQ],
                    start=True,
                    stop=True,
                )
                nc.vector.tensor_copy(
                    out=kt64[64:128, n * BQ : (n + 1) * BQ],
                    in_=shk[64:128, 0, :],
                )

            # ---- transpose back (bf16), normalize (f32), store ----
            oa = oaugp.tile([65, BQ], BF16, tag="oa")
            nc.vector.tensor_copy(out=oa, in_=av)
            tpt = psC.tile([P, 2, BQ], F32, tag="psc")
            tp = tpt[:, 0, 0:288].bitcast(BF16).rearrange(
                "p (q c) -> p q c", c=144
            )
            for q in range(4):
                nc.tensor.transpose(
                    out=tp[:, q, 0:65],
                    in_=oa[:, q * P : (q + 1) * P],
                    identity=identb_sb[0:65, 0:65],
                )
            r = recp.tile([P, 4], F32, tag="r")
            nc.vector.reciprocal(r, tp[:, :, 64])
            ot = outp.tile([P, 4, H], F32, tag="ot")
            for q in range(4):
                nc.vector.tensor_scalar_mul(
                    ot[:, q, :], tp[:, q, 0:64], r[:, q : q + 1]
                )
            # SBUF->DRAM store on the sync HWDGE queue (free after the x
            # transposes; keeps the serial SWDGE queue for the qt64/kt64
            # duplicates so they don't park behind these stores).
            nc.sync.dma_start(
                out=o_d[n * BQ : (n + 1) * BQ, :].rearrange(
                    "(q p) h -> p q h", p=P
                ),
                in_=ot,
            )


def host_inputs(Wq, Wk, Wv):
    """Replicated per-core constant inputs from the raw weights."""
    bf = mybir.dt.np(BF16)
    scale = np.float32(1.0 / np.sqrt(np.float32(C)))
    w1 = np.empty((C, 128), dtype=np.float32)
    w1[:, 0:64] = Wk.T
    w1[:, 64:128] = Wq.T * scale
    w2 = np.empty((C, 128), dtype=np.float32)
    w2[:, 0:64] = Wq.T * scale
    w2[:, 64:128] = Wv.T
    p = np.arange(P, dtype=np.int64)[:, None]
    j = np.arange(P, dtype=np.int64)[None, :]
    masks = np.where(p <= j, 0.0, -1000.0).astype(np.float32)
    identb = np.concatenate(
        [np.eye(P, dtype=np.float32), np.eye(P, P, 64, dtype=np.float32)],
        axis=1,
    )
    return (
        w1.astype(bf),
        w2.astype(bf),
        masks.astype(bf),
        identb.astype(bf),
        np.eye(P, dtype=np.float32),
    )


def kernel(x, Wq, Wk, Wv):
    x = np.asarray(x, dtype=np.float32)
    Wq = np.asarray(Wq, dtype=np.float32)
    Wk = np.asarray(Wk, dtype=np.float32)
    Wv = np.asarray(Wv, dtype=np.float32)
    assert x.shape == (B, T, C), x.shape

    bf = mybir.dt.np(BF16)
    xb = np.ascontiguousarray(x.astype(bf))
    w1, w2, masks, identb, ident = host_inputs(Wq, Wk, Wv)
    nc = build_nc()
    in_maps = [
        {
            "x": np.ascontiguousarray(xb[b]),
            "w1": w1,
            "w2": w2,
            "masks": masks,
            "identb": identb,
            "ident": ident,
        }
        for b in range(B)
    ]
    try:
        res = run_bass_kernel_spmd(nc, in_maps, core_ids=list(range(B)))
    except Exception:
        # transient device/mesh hiccups happen through the tunnel; one retry
        res = run_bass_kernel_spmd(nc, in_maps, core_ids=list(range(B)))
    return np.stack([res.results[b]["out"] for b in range(B)], axis=0)
